# revision 6
# baseline (speedup 1.0000x reference)
"""Trainium2 Bass kernel for nn_ColorLoss (chamfer-style nearest-color loss).

Computation: for each predicted color p (B=2, M=65536, C=3), the euclidean
distance to the nearest gt color (B=2, N=32768, 3) within its batch, then the
mean over all B*M predictions.

Sharding: pred points are split across the 8 cores (B*M/8 = 16384 per core);
each core gets the full gt set of its batch (cores 0-3 -> batch 0, 4-7 ->
batch 1). Each core returns the SUM of its 16384 min-distances; the host
divides by B*M.

Per-core algorithm (brute force, V1):
  For pred m and gt n:  d2[m,n] = |p|^2 + |g|^2 - 2 p.g
  s[m,n] := p.g - |g|^2/2 computed as a K=4 matmul:
      lhsT = [px,py,pz,1] (4 x 128 pred block), rhs = [gx,gy,gz,-|g|^2/2]
  min_n d2 = |p|^2 - 2*max_n s  ->  dist = sqrt(psq - 2*smax), then sum.
  PE streams s into PSUM [128, 2048] tiles; DVE max-reduces each tile.
"""

import numpy as np

import concourse.bacc as bacc
import concourse.tile as tile
from concourse import mybir
from concourse.bass_utils import run_bass_kernel_spmd

B = 2
M_TOTAL = 65536  # preds per batch
N_GT = 32768  # gt per batch
N_CORES = 8
M_CORE = B * M_TOTAL // N_CORES  # 16384 preds per core

FP32 = mybir.dt.float32


def build_kernel(blocks=M_CORE // 128, chunks_per_quarter=4, quarters=16):
    """Build the bass module. blocks*128 preds are processed; each pred is
    compared against quarters*chunks_per_quarter*512 gt points."""
    nc = bacc.Bacc("TRN2", target_bir_lowering=False, debug=False,
                   num_devices=N_CORES)

    pred4_d = nc.dram_tensor("pred4", [4, M_CORE], FP32, kind="ExternalInput")
    prednat_d = nc.dram_tensor("prednat", [M_CORE, 3], FP32,
                               kind="ExternalInput")
    gt3_d = nc.dram_tensor("gt3", [3, N_GT], FP32, kind="ExternalInput")
    gtnat_d = nc.dram_tensor("gtnat", [N_GT, 3], FP32, kind="ExternalInput")
    osum_d = nc.dram_tensor("osum", [1, 1], FP32, kind="ExternalOutput")

    n_pred_blocks = M_CORE // 128  # 128

    with tile.TileContext(nc) as tc:
        with (
            tc.tile_pool(name="const", bufs=1) as const,
            tc.tile_pool(name="prep", bufs=1) as prep,
            tc.tile_pool(name="dram", bufs=1, space="DRAM") as dram,
            tc.tile_pool(name="qmaxp", bufs=3) as qmaxp,
            tc.tile_pool(name="psum", bufs=2, space="PSUM") as psump,
        ):
            # --- load pred lhsT [4, 16384] (x, y, z, 1 rows) ---
            pred4_s = const.tile([4, M_CORE], FP32)
            nc.sync.dma_start(out=pred4_s, in_=pred4_d.ap())

            # --- assemble gt rhs [4, 32768]: rows 0-2 = g, row 3 = -|g|^2/2
            gt4_s = const.tile([4, N_GT], FP32)
            nc.sync.dma_start(out=gt4_s[0:3, :], in_=gt3_d.ap())
            # g2 in natural layout: g = p*256 + blk (sequential when
            # iterated partition-major)
            gtn = prep.tile([128, N_GT // 128, 3], FP32)
            nc.sync.dma_start(
                out=gtn,
                in_=gtnat_d.ap().rearrange("(p blk) c -> p blk c", p=128))
            gsq = prep.tile([128, N_GT // 128, 3], FP32)
            nc.vector.tensor_mul(gsq, gtn, gtn)
            g2n = prep.tile([128, N_GT // 128], FP32)
            nc.vector.tensor_reduce(g2n, gsq, axis=mybir.AxisListType.X,
                                    op=mybir.AluOpType.add)
            g2s = prep.tile([128, N_GT // 128], FP32)
            nc.scalar.mul(g2s, g2n, -0.5)
            # bounce through DRAM to transpose [128, 256] -> [1, 32768]
            g2_dram = dram.tile([128, N_GT // 128], FP32)
            nc.sync.dma_start(out=g2_dram, in_=g2s)
            nc.sync.dma_start(
                out=gt4_s[3:4, :],
                in_=g2_dram.rearrange("(o p) blk -> o (p blk)", o=1))

            # --- psq [128, blocks]: |p|^2, column = pred block, m = blk*128+p
            pn = prep.tile([128, n_pred_blocks, 3], FP32)
            nc.sync.dma_start(
                out=pn,
                in_=prednat_d.ap().rearrange("(blk p) c -> p blk c", p=128))
            psq3 = prep.tile([128, n_pred_blocks, 3], FP32)
            nc.vector.tensor_mul(psq3, pn, pn)
            psq_s = const.tile([128, n_pred_blocks], FP32)
            nc.vector.tensor_reduce(psq_s, psq3, axis=mybir.AxisListType.X,
                                    op=mybir.AluOpType.add)

            ones_s = const.tile([128, 1], FP32)
            nc.vector.memset(ones_s, 1.0)

            smax_all = const.tile([128, n_pred_blocks], FP32)

            # --- main loop ---
            qwidth = chunks_per_quarter * 512
            for blk in range(blocks):
                lhsT = pred4_s[:, blk * 128:(blk + 1) * 128]
                qmax = qmaxp.tile([128, quarters], FP32)
                for q in range(quarters):
                    ps = psump.tile([128, qwidth], FP32)
                    for k in range(chunks_per_quarter):
                        n0 = (q * chunks_per_quarter + k) * 512
                        nc.tensor.matmul(ps[:, k * 512:(k + 1) * 512], lhsT,
                                         gt4_s[:, n0:n0 + 512],
                                         start=True, stop=True)
                    nc.vector.tensor_reduce(qmax[:, q:q + 1], ps,
                                            axis=mybir.AxisListType.X,
                                            op=mybir.AluOpType.max)
                nc.vector.tensor_reduce(smax_all[:, blk:blk + 1], qmax,
                                        axis=mybir.AxisListType.X,
                                        op=mybir.AluOpType.max)

            # --- dist = sqrt(max(psq - 2*smax, 0)); partial sum ---
            dsq = prep.tile([128, n_pred_blocks], FP32)
            nc.vector.scalar_tensor_tensor(
                out=dsq[:, 0:blocks], in0=smax_all[:, 0:blocks], scalar=-2.0,
                in1=psq_s[:, 0:blocks],
                op0=mybir.AluOpType.mult, op1=mybir.AluOpType.add)
            dsqc = prep.tile([128, n_pred_blocks], FP32)
            nc.vector.tensor_scalar_max(dsqc[:, 0:blocks], dsq[:, 0:blocks],
                                        0.0)
            dist = prep.tile([128, n_pred_blocks], FP32)
            nc.scalar.activation(dist[:, 0:blocks], dsqc[:, 0:blocks],
                                 func=mybir.ActivationFunctionType.Sqrt)
            rowsum = prep.tile([128, 1], FP32)
            nc.vector.tensor_reduce(rowsum, dist[:, 0:blocks],
                                    axis=mybir.AxisListType.X,
                                    op=mybir.AluOpType.add)
            # cross-partition sum via K=128 matmul with ones
            pst = psump.tile([128, qwidth], FP32, tag="ps")
            nc.tensor.matmul(pst[0:1, 0:1], ones_s, rowsum,
                             start=True, stop=True)
            out_s = prep.tile([1, 1], FP32)
            nc.vector.tensor_copy(out_s, pst[0:1, 0:1])
            nc.sync.dma_start(out=osum_d.ap(), in_=out_s)

    nc.compile()
    return nc


def build_kernel_loop(blocks=M_CORE // 128, chunks_per_quarter=4, quarters=16):
    """Same computation as build_kernel, but the 128-block loop is a hardware
    For_i loop (program ~110 instructions instead of ~10k => much faster
    neuronxcc compile). lhsT is staged into a fixed SBUF tile each iteration
    because ldweights cannot take register offsets."""
    from concourse.bass import ds

    nc = bacc.Bacc("TRN2", target_bir_lowering=False, debug=False,
                   num_devices=N_CORES)

    pred4_d = nc.dram_tensor("pred4", [4, M_CORE], FP32, kind="ExternalInput")
    prednat_d = nc.dram_tensor("prednat", [M_CORE, 3], FP32,
                               kind="ExternalInput")
    gt3_d = nc.dram_tensor("gt3", [3, N_GT], FP32, kind="ExternalInput")
    gtnat_d = nc.dram_tensor("gtnat", [N_GT, 3], FP32, kind="ExternalInput")
    osum_d = nc.dram_tensor("osum", [1, 1], FP32, kind="ExternalOutput")

    n_pred_blocks = M_CORE // 128

    with tile.TileContext(nc) as tc:
        with (
            tc.tile_pool(name="const", bufs=1) as const,
            tc.tile_pool(name="prep", bufs=1) as prep,
            tc.tile_pool(name="dram", bufs=1, space="DRAM") as dram,
            tc.tile_pool(name="loopp", bufs=2) as loopp,
            tc.tile_pool(name="psum", bufs=2, space="PSUM") as psump,
        ):
            # --- setup (identical to build_kernel) ---
            pred4_s = const.tile([4, M_CORE], FP32)
            nc.sync.dma_start(out=pred4_s, in_=pred4_d.ap())

            gt4_s = const.tile([4, N_GT], FP32)
            nc.sync.dma_start(out=gt4_s[0:3, :], in_=gt3_d.ap())
            gtn = prep.tile([128, N_GT // 128, 3], FP32)
            nc.sync.dma_start(
                out=gtn,
                in_=gtnat_d.ap().rearrange("(p blk) c -> p blk c", p=128))
            gsq = prep.tile([128, N_GT // 128, 3], FP32)
            nc.vector.tensor_mul(gsq, gtn, gtn)
            g2n = prep.tile([128, N_GT // 128], FP32)
            nc.vector.tensor_reduce(g2n, gsq, axis=mybir.AxisListType.X,
                                    op=mybir.AluOpType.add)
            g2s = prep.tile([128, N_GT // 128], FP32)
            nc.scalar.mul(g2s, g2n, -0.5)
            g2_dram = dram.tile([128, N_GT // 128], FP32)
            nc.sync.dma_start(out=g2_dram, in_=g2s)
            nc.sync.dma_start(
                out=gt4_s[3:4, :],
                in_=g2_dram.rearrange("(o p) blk -> o (p blk)", o=1))

            pn = prep.tile([128, n_pred_blocks, 3], FP32)
            nc.sync.dma_start(
                out=pn,
                in_=prednat_d.ap().rearrange("(blk p) c -> p blk c", p=128))
            psq3 = prep.tile([128, n_pred_blocks, 3], FP32)
            nc.vector.tensor_mul(psq3, pn, pn)
            psq_s = const.tile([128, n_pred_blocks], FP32)
            nc.vector.tensor_reduce(psq_s, psq3, axis=mybir.AxisListType.X,
                                    op=mybir.AluOpType.add)

            ones_s = const.tile([128, 1], FP32)
            nc.vector.memset(ones_s, 1.0)
            sumacc = const.tile([128, 1], FP32)
            nc.vector.memset(sumacc, 0.0)

            # --- main hardware loop over pred blocks ---
            qwidth = chunks_per_quarter * 512
            with tc.For_i(0, blocks, 1) as blk:
                lhsT_f = loopp.tile([4, 128], FP32, tag="lhsT")
                nc.vector.tensor_copy(lhsT_f,
                                      pred4_s[:, ds(blk * 128, 128)])
                qmax = loopp.tile([128, quarters], FP32, tag="qmax")
                for q in range(quarters):
                    ps = psump.tile([128, qwidth], FP32, tag="ps")
                    for k in range(chunks_per_quarter):
                        n0 = (q * chunks_per_quarter + k) * 512
                        nc.tensor.matmul(ps[:, k * 512:(k + 1) * 512],
                                         lhsT_f, gt4_s[:, n0:n0 + 512],
                                         start=True, stop=True)
                    nc.vector.tensor_reduce(qmax[:, q:q + 1], ps,
                                            axis=mybir.AxisListType.X,
                                            op=mybir.AluOpType.max)
                smax_c = loopp.tile([128, 1], FP32, tag="smax")
                nc.vector.tensor_reduce(smax_c, qmax,
                                        axis=mybir.AxisListType.X,
                                        op=mybir.AluOpType.max)
                # dsq = psq[:, blk] - 2*smax ; clamp ; sqrt ; accumulate
                dsq_c = loopp.tile([128, 1], FP32, tag="dsq")
                nc.vector.scalar_tensor_tensor(
                    out=dsq_c, in0=smax_c, scalar=-2.0,
                    in1=psq_s[:, ds(blk, 1)],
                    op0=mybir.AluOpType.mult, op1=mybir.AluOpType.add)
                dsqc_c = loopp.tile([128, 1], FP32, tag="dsqc")
                nc.vector.tensor_scalar_max(dsqc_c, dsq_c, 0.0)
                dist_c = loopp.tile([128, 1], FP32, tag="dist")
                nc.scalar.activation(dist_c, dsqc_c,
                                     func=mybir.ActivationFunctionType.Sqrt)
                nc.vector.tensor_add(sumacc, sumacc, dist_c)

            # --- final cross-partition sum ---
            pst = psump.tile([128, qwidth], FP32, tag="ps")
            nc.tensor.matmul(pst[0:1, 0:1], ones_s, sumacc,
                             start=True, stop=True)
            out_s = prep.tile([1, 1], FP32)
            nc.vector.tensor_copy(out_s, pst[0:1, 0:1])
            nc.sync.dma_start(out=osum_d.ap(), in_=out_s)

    nc.compile()
    return nc


def build_baseline():
    """Trivial kernel with identical I/O signature, for dispatch-overhead
    baseline measurement in test.py."""
    nc = bacc.Bacc("TRN2", target_bir_lowering=False, debug=False,
                   num_devices=N_CORES)
    pred4_d = nc.dram_tensor("pred4", [4, M_CORE], FP32, kind="ExternalInput")
    nc.dram_tensor("prednat", [M_CORE, 3], FP32, kind="ExternalInput")
    nc.dram_tensor("gt3", [3, N_GT], FP32, kind="ExternalInput")
    nc.dram_tensor("gtnat", [N_GT, 3], FP32, kind="ExternalInput")
    osum_d = nc.dram_tensor("osum", [1, 1], FP32, kind="ExternalOutput")
    with tile.TileContext(nc) as tc:
        with tc.tile_pool(name="p", bufs=1) as pool:
            t = pool.tile([1, 1], FP32)
            nc.sync.dma_start(out=t, in_=pred4_d.ap()[0:1, 0:1])
            nc.sync.dma_start(out=osum_d.ap(), in_=t)
    nc.compile()
    return nc


def _make_in_maps(pred_colors, gt_colors):
    in_maps = []
    for c in range(N_CORES):
        b = c // (N_CORES // B)
        sl = c % (N_CORES // B)
        pred_slice = np.ascontiguousarray(
            pred_colors[b, sl * M_CORE:(sl + 1) * M_CORE]).astype(
                np.float32, copy=False)
        pred4 = np.empty((4, M_CORE), np.float32)
        pred4[0:3] = pred_slice.T
        pred4[3] = 1.0
        gt_b = np.ascontiguousarray(gt_colors[b]).astype(np.float32,
                                                         copy=False)
        gt3 = np.ascontiguousarray(gt_b.T)
        in_maps.append({
            "pred4": pred4,
            "prednat": pred_slice,
            "gt3": gt3,
            "gtnat": gt_b,
        })
    return in_maps


_NC_CACHE = {}


def kernel(pred_colors: np.ndarray, gt_colors: np.ndarray) -> np.ndarray:
    pred_colors = np.asarray(pred_colors)
    gt_colors = np.asarray(gt_colors)
    assert pred_colors.shape == (B, M_TOTAL, 3)
    assert gt_colors.shape == (B, N_GT, 3)

    if "nc" not in _NC_CACHE:
        _NC_CACHE["nc"] = build_kernel()
    nc = _NC_CACHE["nc"]

    in_maps = _make_in_maps(pred_colors, gt_colors)
    res = run_bass_kernel_spmd(nc, in_maps, core_ids=list(range(N_CORES)),
                               trace=False)
    total = np.float64(0.0)
    for c in range(N_CORES):
        total += np.float64(res.results[c]["osum"][0, 0])
    mean = np.float32(total / (B * M_TOTAL))
    return np.asarray(mean, dtype=np.float32)


if __name__ == "__main__":
    rng = np.random.default_rng(0)
    pred = rng.random((B, M_TOTAL, 3), dtype=np.float32)
    gt = rng.random((B, N_GT, 3), dtype=np.float32)
    out = kernel(pred, gt)
    print("kernel out:", out)


# revision 12
# speedup vs baseline: 64.4997x; 64.4997x over previous
"""Trainium2 Bass kernel for nn_ColorLoss (chamfer-style nearest-color loss).

Computation: for each predicted color p (B=2, M=65536, C=3), the euclidean
distance to the nearest gt color (B=2, N=32768, 3) within its batch, then the
mean over all B*M predictions.

Sharding: pred points are split across the 8 cores (B*M/8 = 16384 per core);
each core gets the full gt set of its batch (cores 0-3 -> batch 0, 4-7 ->
batch 1). Each core returns the SUM of its 16384 min-distances; the host
divides by B*M.

Per-core algorithm (brute force, V1):
  For pred m and gt n:  d2[m,n] = |p|^2 + |g|^2 - 2 p.g
  s[m,n] := p.g - |g|^2/2 computed as a K=4 matmul:
      lhsT = [px,py,pz,1] (4 x 128 pred block), rhs = [gx,gy,gz,-|g|^2/2]
  min_n d2 = |p|^2 - 2*max_n s  ->  dist = sqrt(psq - 2*smax), then sum.
  PE streams s into PSUM [128, 2048] tiles; DVE max-reduces each tile.
"""

import numpy as np

import concourse.bacc as bacc
import concourse.tile as tile
from concourse import mybir
from concourse.bass_utils import run_bass_kernel_spmd

B = 2
M_TOTAL = 65536  # preds per batch
N_GT = 32768  # gt per batch
N_CORES = 8
M_CORE = B * M_TOTAL // N_CORES  # 16384 preds per core

FP32 = mybir.dt.float32


def build_kernel(blocks=M_CORE // 128, chunks_per_quarter=4, quarters=16):
    """Build the bass module. blocks*128 preds are processed; each pred is
    compared against quarters*chunks_per_quarter*512 gt points."""
    nc = bacc.Bacc("TRN2", target_bir_lowering=False, debug=False,
                   num_devices=N_CORES)

    pred4_d = nc.dram_tensor("pred4", [4, M_CORE], FP32, kind="ExternalInput")
    prednat_d = nc.dram_tensor("prednat", [M_CORE, 3], FP32,
                               kind="ExternalInput")
    gt3_d = nc.dram_tensor("gt3", [3, N_GT], FP32, kind="ExternalInput")
    gtnat_d = nc.dram_tensor("gtnat", [N_GT, 3], FP32, kind="ExternalInput")
    osum_d = nc.dram_tensor("osum", [1, 1], FP32, kind="ExternalOutput")

    n_pred_blocks = M_CORE // 128  # 128

    with tile.TileContext(nc) as tc:
        with (
            tc.tile_pool(name="const", bufs=1) as const,
            tc.tile_pool(name="prep", bufs=1) as prep,
            tc.tile_pool(name="dram", bufs=1, space="DRAM") as dram,
            tc.tile_pool(name="qmaxp", bufs=3) as qmaxp,
            tc.tile_pool(name="psum", bufs=2, space="PSUM") as psump,
        ):
            # --- load pred lhsT [4, 16384] (x, y, z, 1 rows) ---
            pred4_s = const.tile([4, M_CORE], FP32)
            nc.sync.dma_start(out=pred4_s, in_=pred4_d.ap())

            # --- assemble gt rhs [4, 32768]: rows 0-2 = g, row 3 = -|g|^2/2
            gt4_s = const.tile([4, N_GT], FP32)
            nc.sync.dma_start(out=gt4_s[0:3, :], in_=gt3_d.ap())
            # g2 in natural layout: g = p*256 + blk (sequential when
            # iterated partition-major)
            gtn = prep.tile([128, N_GT // 128, 3], FP32)
            nc.sync.dma_start(
                out=gtn,
                in_=gtnat_d.ap().rearrange("(p blk) c -> p blk c", p=128))
            gsq = prep.tile([128, N_GT // 128, 3], FP32)
            nc.vector.tensor_mul(gsq, gtn, gtn)
            g2n = prep.tile([128, N_GT // 128], FP32)
            nc.vector.tensor_reduce(g2n, gsq, axis=mybir.AxisListType.X,
                                    op=mybir.AluOpType.add)
            g2s = prep.tile([128, N_GT // 128], FP32)
            nc.scalar.mul(g2s, g2n, -0.5)
            # bounce through DRAM to transpose [128, 256] -> [1, 32768]
            g2_dram = dram.tile([128, N_GT // 128], FP32)
            nc.sync.dma_start(out=g2_dram, in_=g2s)
            nc.sync.dma_start(
                out=gt4_s[3:4, :],
                in_=g2_dram.rearrange("(o p) blk -> o (p blk)", o=1))

            # --- psq [128, blocks]: |p|^2, column = pred block, m = blk*128+p
            pn = prep.tile([128, n_pred_blocks, 3], FP32)
            nc.sync.dma_start(
                out=pn,
                in_=prednat_d.ap().rearrange("(blk p) c -> p blk c", p=128))
            psq3 = prep.tile([128, n_pred_blocks, 3], FP32)
            nc.vector.tensor_mul(psq3, pn, pn)
            psq_s = const.tile([128, n_pred_blocks], FP32)
            nc.vector.tensor_reduce(psq_s, psq3, axis=mybir.AxisListType.X,
                                    op=mybir.AluOpType.add)

            ones_s = const.tile([128, 1], FP32)
            nc.vector.memset(ones_s, 1.0)

            smax_all = const.tile([128, n_pred_blocks], FP32)

            # --- main loop ---
            qwidth = chunks_per_quarter * 512
            for blk in range(blocks):
                lhsT = pred4_s[:, blk * 128:(blk + 1) * 128]
                qmax = qmaxp.tile([128, quarters], FP32)
                for q in range(quarters):
                    ps = psump.tile([128, qwidth], FP32)
                    for k in range(chunks_per_quarter):
                        n0 = (q * chunks_per_quarter + k) * 512
                        nc.tensor.matmul(ps[:, k * 512:(k + 1) * 512], lhsT,
                                         gt4_s[:, n0:n0 + 512],
                                         start=True, stop=True)
                    nc.vector.tensor_reduce(qmax[:, q:q + 1], ps,
                                            axis=mybir.AxisListType.X,
                                            op=mybir.AluOpType.max)
                nc.vector.tensor_reduce(smax_all[:, blk:blk + 1], qmax,
                                        axis=mybir.AxisListType.X,
                                        op=mybir.AluOpType.max)

            # --- dist = sqrt(max(psq - 2*smax, 0)); partial sum ---
            dsq = prep.tile([128, n_pred_blocks], FP32)
            nc.vector.scalar_tensor_tensor(
                out=dsq[:, 0:blocks], in0=smax_all[:, 0:blocks], scalar=-2.0,
                in1=psq_s[:, 0:blocks],
                op0=mybir.AluOpType.mult, op1=mybir.AluOpType.add)
            dsqc = prep.tile([128, n_pred_blocks], FP32)
            nc.vector.tensor_scalar_max(dsqc[:, 0:blocks], dsq[:, 0:blocks],
                                        0.0)
            dist = prep.tile([128, n_pred_blocks], FP32)
            nc.scalar.activation(dist[:, 0:blocks], dsqc[:, 0:blocks],
                                 func=mybir.ActivationFunctionType.Sqrt)
            rowsum = prep.tile([128, 1], FP32)
            nc.vector.tensor_reduce(rowsum, dist[:, 0:blocks],
                                    axis=mybir.AxisListType.X,
                                    op=mybir.AluOpType.add)
            # cross-partition sum via K=128 matmul with ones
            pst = psump.tile([128, qwidth], FP32, tag="ps")
            nc.tensor.matmul(pst[0:1, 0:1], ones_s, rowsum,
                             start=True, stop=True)
            out_s = prep.tile([1, 1], FP32)
            nc.vector.tensor_copy(out_s, pst[0:1, 0:1])
            nc.sync.dma_start(out=osum_d.ap(), in_=out_s)

    nc.compile()
    return nc


def build_kernel_loop(blocks=M_CORE // 128, chunks_per_quarter=4, quarters=16):
    """Same computation as build_kernel, but the 128-block loop is a hardware
    For_i loop (program ~110 instructions instead of ~10k => much faster
    neuronxcc compile). lhsT is staged into a fixed SBUF tile each iteration
    because ldweights cannot take register offsets."""
    from concourse.bass import ds

    nc = bacc.Bacc("TRN2", target_bir_lowering=False, debug=False,
                   num_devices=N_CORES)

    pred4_d = nc.dram_tensor("pred4", [4, M_CORE], FP32, kind="ExternalInput")
    prednat_d = nc.dram_tensor("prednat", [M_CORE, 3], FP32,
                               kind="ExternalInput")
    gt3_d = nc.dram_tensor("gt3", [3, N_GT], FP32, kind="ExternalInput")
    gtnat_d = nc.dram_tensor("gtnat", [N_GT, 3], FP32, kind="ExternalInput")
    osum_d = nc.dram_tensor("osum", [1, 1], FP32, kind="ExternalOutput")

    n_pred_blocks = M_CORE // 128

    with tile.TileContext(nc) as tc:
        with (
            tc.tile_pool(name="const", bufs=1) as const,
            tc.tile_pool(name="prep", bufs=1) as prep,
            tc.tile_pool(name="dram", bufs=1, space="DRAM") as dram,
            tc.tile_pool(name="loopp", bufs=2) as loopp,
            tc.tile_pool(name="psum", bufs=2, space="PSUM") as psump,
        ):
            # --- setup (identical to build_kernel) ---
            pred4_s = const.tile([4, M_CORE], FP32)
            nc.sync.dma_start(out=pred4_s, in_=pred4_d.ap())

            gt4_s = const.tile([4, N_GT], FP32)
            nc.sync.dma_start(out=gt4_s[0:3, :], in_=gt3_d.ap())
            gtn = prep.tile([128, N_GT // 128, 3], FP32)
            nc.sync.dma_start(
                out=gtn,
                in_=gtnat_d.ap().rearrange("(p blk) c -> p blk c", p=128))
            gsq = prep.tile([128, N_GT // 128, 3], FP32)
            nc.vector.tensor_mul(gsq, gtn, gtn)
            g2n = prep.tile([128, N_GT // 128], FP32)
            nc.vector.tensor_reduce(g2n, gsq, axis=mybir.AxisListType.X,
                                    op=mybir.AluOpType.add)
            g2s = prep.tile([128, N_GT // 128], FP32)
            nc.scalar.mul(g2s, g2n, -0.5)
            g2_dram = dram.tile([128, N_GT // 128], FP32)
            nc.sync.dma_start(out=g2_dram, in_=g2s)
            nc.sync.dma_start(
                out=gt4_s[3:4, :],
                in_=g2_dram.rearrange("(o p) blk -> o (p blk)", o=1))

            pn = prep.tile([128, n_pred_blocks, 3], FP32)
            nc.sync.dma_start(
                out=pn,
                in_=prednat_d.ap().rearrange("(blk p) c -> p blk c", p=128))
            psq3 = prep.tile([128, n_pred_blocks, 3], FP32)
            nc.vector.tensor_mul(psq3, pn, pn)
            psq_s = const.tile([128, n_pred_blocks], FP32)
            nc.vector.tensor_reduce(psq_s, psq3, axis=mybir.AxisListType.X,
                                    op=mybir.AluOpType.add)

            ones_s = const.tile([128, 1], FP32)
            nc.vector.memset(ones_s, 1.0)
            sumacc = const.tile([128, 1], FP32)
            nc.vector.memset(sumacc, 0.0)

            # --- main hardware loop over pred blocks ---
            qwidth = chunks_per_quarter * 512
            with tc.For_i(0, blocks, 1) as blk:
                lhsT_f = loopp.tile([4, 128], FP32, tag="lhsT")
                nc.vector.tensor_copy(lhsT_f,
                                      pred4_s[:, ds(blk * 128, 128)])
                qmax = loopp.tile([128, quarters], FP32, tag="qmax")
                for q in range(quarters):
                    ps = psump.tile([128, qwidth], FP32, tag="ps")
                    for k in range(chunks_per_quarter):
                        n0 = (q * chunks_per_quarter + k) * 512
                        nc.tensor.matmul(ps[:, k * 512:(k + 1) * 512],
                                         lhsT_f, gt4_s[:, n0:n0 + 512],
                                         start=True, stop=True)
                    nc.vector.tensor_reduce(qmax[:, q:q + 1], ps,
                                            axis=mybir.AxisListType.X,
                                            op=mybir.AluOpType.max)
                smax_c = loopp.tile([128, 1], FP32, tag="smax")
                nc.vector.tensor_reduce(smax_c, qmax,
                                        axis=mybir.AxisListType.X,
                                        op=mybir.AluOpType.max)
                # dsq = psq[:, blk] - 2*smax ; clamp ; sqrt ; accumulate
                dsq_c = loopp.tile([128, 1], FP32, tag="dsq")
                nc.vector.scalar_tensor_tensor(
                    out=dsq_c, in0=smax_c, scalar=-2.0,
                    in1=psq_s[:, ds(blk, 1)],
                    op0=mybir.AluOpType.mult, op1=mybir.AluOpType.add)
                dsqc_c = loopp.tile([128, 1], FP32, tag="dsqc")
                nc.vector.tensor_scalar_max(dsqc_c, dsq_c, 0.0)
                dist_c = loopp.tile([128, 1], FP32, tag="dist")
                nc.scalar.activation(dist_c, dsqc_c,
                                     func=mybir.ActivationFunctionType.Sqrt)
                nc.vector.tensor_add(sumacc, sumacc, dist_c)

            # --- final cross-partition sum ---
            pst = psump.tile([128, qwidth], FP32, tag="ps")
            nc.tensor.matmul(pst[0:1, 0:1], ones_s, sumacc,
                             start=True, stop=True)
            out_s = prep.tile([1, 1], FP32)
            nc.vector.tensor_copy(out_s, pst[0:1, 0:1])
            nc.sync.dma_start(out=osum_d.ap(), in_=out_s)

    nc.compile()
    return nc


BF16 = mybir.dt.bfloat16


def build_kernel_loop_bf16(blocks=M_CORE // 128, chunks_per_quarter=4,
                           quarters=16):
    """Loop kernel with the fp32 matmul replaced by ONE bf16 matmul of K=21
    per 512-chunk. p and g are split into 3 bf16 levels (hi/lo/lo2); all
    product terms >= ~2^-27 are kept by stacking them along the contraction
    dim (K=21), which is free on the PE (cost ~ N columns only):

      k 0-2 : P   x G      k 9-11 : p'  x G      k 18: 1 x -G2/2
      k 3-5 : P   x g'     k 12-14: p'' x G      k 19: 1 x -g2'/2
      k 6-8 : P   x g''    k 15-17: p'  x g'     k 20: 1 x -g2''/2

    |error on s| <= ~1e-7, i.e. fp32-equivalent for this data.
    """
    from concourse.bass import ds

    nc = bacc.Bacc("TRN2", target_bir_lowering=False, debug=False,
                   num_devices=N_CORES)

    prednat_d = nc.dram_tensor("prednat", [M_CORE, 3], FP32,
                               kind="ExternalInput")
    gtnat_d = nc.dram_tensor("gtnat", [N_GT, 3], FP32, kind="ExternalInput")
    osum_d = nc.dram_tensor("osum", [1, 1], FP32, kind="ExternalOutput")

    n_pred_blocks = M_CORE // 128
    NB_GT = N_GT // 128  # 256

    K21 = 21

    with tile.TileContext(nc) as tc:
        with (
            tc.tile_pool(name="const", bufs=1) as const,
            tc.tile_pool(name="prep", bufs=1) as prep,
            tc.tile_pool(name="dram", bufs=1, space="DRAM") as dram,
            tc.tile_pool(name="loopp", bufs=2) as loopp,
            tc.tile_pool(name="psum", bufs=2, space="PSUM") as psump,
        ):
            # ---------- gt natural load (g = p*256 + blk) ----------
            gtn = prep.tile([128, NB_GT, 3], FP32)
            nc.sync.dma_start(
                out=gtn,
                in_=gtnat_d.ap().rearrange("(p blk) c -> p blk c", p=128))
            # g2 = -|g|^2/2 in fp32
            gsq = prep.tile([128, NB_GT, 3], FP32)
            nc.vector.tensor_mul(gsq, gtn, gtn)
            g2f = prep.tile([128, NB_GT], FP32)
            nc.vector.tensor_reduce(g2f, gsq, axis=mybir.AxisListType.X,
                                    op=mybir.AluOpType.add)
            g2s = prep.tile([128, NB_GT], FP32)
            nc.scalar.mul(g2s, g2f, -0.5)

            def split3(src_ap, shape):
                """Return bf16 (hi, lo, lo2) tiles for fp32 src_ap."""
                hi = prep.tile(shape, BF16)
                nc.vector.tensor_copy(hi, src_ap)
                r1 = prep.tile(shape, FP32)
                nc.vector.tensor_sub(r1, src_ap, hi)
                lo = prep.tile(shape, BF16)
                nc.vector.tensor_copy(lo, r1)
                r2 = prep.tile(shape, FP32)
                nc.vector.tensor_sub(r2, r1, lo)
                lo2 = prep.tile(shape, BF16)
                nc.vector.tensor_copy(lo2, r2)
                return hi, lo, lo2

            ghi, glo, glo2 = split3(gtn, [128, NB_GT, 3])
            g2hi, g2lo, g2lo2 = split3(g2s, [128, NB_GT])

            # bounce to DRAM for transposed assembly
            def to_dram(t, shape):
                d = dram.tile(shape, BF16)
                nc.sync.dma_start(out=d, in_=t)
                return d

            ghi_d = to_dram(ghi, [128, NB_GT, 3])
            glo_d = to_dram(glo, [128, NB_GT, 3])
            glo2_d = to_dram(glo2, [128, NB_GT, 3])
            g2hi_d = to_dram(g2hi, [128, NB_GT])
            g2lo_d = to_dram(g2lo, [128, NB_GT])
            g2lo2_d = to_dram(g2lo2, [128, NB_GT])

            # gt rhs [21, 32768] bf16
            gt21 = const.tile([K21, N_GT], BF16)

            def row_from(dram3, col, dst_row):
                # dram3 [128, NB, 3] -> [1, N_GT] taking component `col`,
                # g-major order
                src = dram3.rearrange("p blk c -> c (p blk)")[col:col + 1, :]
                nc.sync.dma_start(out=gt21[dst_row:dst_row + 1, :], in_=src)

            def row_from2(dram2, dst_row):
                src = dram2.rearrange("(o p) blk -> o (p blk)", o=1)
                nc.sync.dma_start(out=gt21[dst_row:dst_row + 1, :], in_=src)

            for c in range(3):
                row_from(ghi_d, c, 0 + c)      # G   (vs P)
                row_from(glo_d, c, 3 + c)      # g'  (vs P)
                row_from(glo2_d, c, 6 + c)     # g'' (vs P)
                row_from(ghi_d, c, 9 + c)      # G   (vs p')
                row_from(ghi_d, c, 12 + c)     # G   (vs p'')
                row_from(glo_d, c, 15 + c)     # g'  (vs p')
            row_from2(g2hi_d, 18)
            row_from2(g2lo_d, 19)
            row_from2(g2lo2_d, 20)

            # ---------- pred natural load (m = blk*128 + p) ----------
            pn = prep.tile([128, n_pred_blocks, 3], FP32)
            nc.sync.dma_start(
                out=pn,
                in_=prednat_d.ap().rearrange("(blk p) c -> p blk c", p=128))
            psq3 = prep.tile([128, n_pred_blocks, 3], FP32)
            nc.vector.tensor_mul(psq3, pn, pn)
            psq_s = const.tile([128, n_pred_blocks], FP32)
            nc.vector.tensor_reduce(psq_s, psq3, axis=mybir.AxisListType.X,
                                    op=mybir.AluOpType.add)

            phi, plo, plo2 = split3(pn, [128, n_pred_blocks, 3])
            phi_d = to_dram(phi, [128, n_pred_blocks, 3])
            plo_d = to_dram(plo, [128, n_pred_blocks, 3])
            plo2_d = to_dram(plo2, [128, n_pred_blocks, 3])

            # rows 18-20 must be 1.0; memset the whole tile (engines cannot
            # start at partition 18) and let the row DMAs overwrite 0-17
            pred21 = const.tile([K21, M_CORE], BF16)
            nc.vector.memset(pred21, 1.0)

            def prow_from(dram3, col, dst_row):
                # dram3 [128, NBLK, 3], m = blk*128 + p -> m-major needs
                # (blk p) order; strides don't nest contiguously so keep a
                # 3-dim AP [1, NBLK, 128] instead of merging
                src = dram3.rearrange("p blk c -> c blk p")[col:col + 1, :, :]
                nc.sync.dma_start(out=pred21[dst_row:dst_row + 1, :], in_=src)

            for c in range(3):
                prow_from(phi_d, c, 0 + c)     # P
                prow_from(phi_d, c, 3 + c)     # P
                prow_from(phi_d, c, 6 + c)     # P
                prow_from(plo_d, c, 9 + c)     # p'
                prow_from(plo2_d, c, 12 + c)   # p''
                prow_from(plo_d, c, 15 + c)    # p'
            # rows 18-20 = 1.0 (set above)

            ones_s = const.tile([128, 1], FP32)
            nc.vector.memset(ones_s, 1.0)
            sumacc = const.tile([128, 1], FP32)
            nc.vector.memset(sumacc, 0.0)

            # ---------- main hardware loop ----------
            qwidth = chunks_per_quarter * 512
            with tc.For_i(0, blocks, 1) as blk:
                lhsT_f = loopp.tile([K21, 128], BF16, tag="lhsT")
                nc.vector.tensor_copy(lhsT_f,
                                      pred21[:, ds(blk * 128, 128)])
                qmax = loopp.tile([128, quarters], FP32, tag="qmax")
                for q in range(quarters):
                    ps = psump.tile([128, qwidth], FP32, tag="ps")
                    for k in range(chunks_per_quarter):
                        n0 = (q * chunks_per_quarter + k) * 512
                        nc.tensor.matmul(ps[:, k * 512:(k + 1) * 512],
                                         lhsT_f, gt21[:, n0:n0 + 512],
                                         start=True, stop=True)
                    nc.vector.tensor_reduce(qmax[:, q:q + 1], ps,
                                            axis=mybir.AxisListType.X,
                                            op=mybir.AluOpType.max)
                smax_c = loopp.tile([128, 1], FP32, tag="smax")
                nc.vector.tensor_reduce(smax_c, qmax,
                                        axis=mybir.AxisListType.X,
                                        op=mybir.AluOpType.max)
                dsq_c = loopp.tile([128, 1], FP32, tag="dsq")
                nc.vector.scalar_tensor_tensor(
                    out=dsq_c, in0=smax_c, scalar=-2.0,
                    in1=psq_s[:, ds(blk, 1)],
                    op0=mybir.AluOpType.mult, op1=mybir.AluOpType.add)
                dsqc_c = loopp.tile([128, 1], FP32, tag="dsqc")
                nc.vector.tensor_scalar_max(dsqc_c, dsq_c, 0.0)
                dist_c = loopp.tile([128, 1], FP32, tag="dist")
                nc.scalar.activation(dist_c, dsqc_c,
                                     func=mybir.ActivationFunctionType.Sqrt)
                nc.vector.tensor_add(sumacc, sumacc, dist_c)

            pst = psump.tile([128, qwidth], FP32, tag="ps")
            nc.tensor.matmul(pst[0:1, 0:1], ones_s, sumacc,
                             start=True, stop=True)
            out_s = prep.tile([1, 1], FP32)
            nc.vector.tensor_copy(out_s, pst[0:1, 0:1])
            nc.sync.dma_start(out=osum_d.ap(), in_=out_s)

    nc.compile()
    return nc


def build_baseline():
    """Trivial kernel with identical I/O signature, for dispatch-overhead
    baseline measurement in test.py."""
    nc = bacc.Bacc("TRN2", target_bir_lowering=False, debug=False,
                   num_devices=N_CORES)
    pred4_d = nc.dram_tensor("pred4", [4, M_CORE], FP32, kind="ExternalInput")
    nc.dram_tensor("prednat", [M_CORE, 3], FP32, kind="ExternalInput")
    nc.dram_tensor("gt3", [3, N_GT], FP32, kind="ExternalInput")
    nc.dram_tensor("gtnat", [N_GT, 3], FP32, kind="ExternalInput")
    osum_d = nc.dram_tensor("osum", [1, 1], FP32, kind="ExternalOutput")
    with tile.TileContext(nc) as tc:
        with tc.tile_pool(name="p", bufs=1) as pool:
            t = pool.tile([1, 1], FP32)
            nc.sync.dma_start(out=t, in_=pred4_d.ap()[0:1, 0:1])
            nc.sync.dma_start(out=osum_d.ap(), in_=t)
    nc.compile()
    return nc


def _make_in_maps(pred_colors, gt_colors):
    in_maps = []
    for c in range(N_CORES):
        b = c // (N_CORES // B)
        sl = c % (N_CORES // B)
        pred_slice = np.ascontiguousarray(
            pred_colors[b, sl * M_CORE:(sl + 1) * M_CORE]).astype(
                np.float32, copy=False)
        pred4 = np.empty((4, M_CORE), np.float32)
        pred4[0:3] = pred_slice.T
        pred4[3] = 1.0
        gt_b = np.ascontiguousarray(gt_colors[b]).astype(np.float32,
                                                         copy=False)
        gt3 = np.ascontiguousarray(gt_b.T)
        in_maps.append({
            "pred4": pred4,
            "prednat": pred_slice,
            "gt3": gt3,
            "gtnat": gt_b,
        })
    return in_maps


_NC_CACHE = {}


def kernel(pred_colors: np.ndarray, gt_colors: np.ndarray) -> np.ndarray:
    pred_colors = np.asarray(pred_colors)
    gt_colors = np.asarray(gt_colors)
    assert pred_colors.shape == (B, M_TOTAL, 3)
    assert gt_colors.shape == (B, N_GT, 3)

    if "nc" not in _NC_CACHE:
        _NC_CACHE["nc"] = build_kernel_loop_bf16()
    nc = _NC_CACHE["nc"]

    in_maps = _make_in_maps(pred_colors, gt_colors)
    # keep only the inputs this kernel flavor declares
    declared = set()
    for alloc in nc.m.functions[0].allocations:
        try:
            if alloc.kind == "ExternalInput" and alloc.memorylocations:
                declared.add(alloc.memorylocations[0].name)
        except AttributeError:
            pass
    in_maps = [{k: v for k, v in m.items() if k in declared}
               for m in in_maps]
    res = run_bass_kernel_spmd(nc, in_maps, core_ids=list(range(N_CORES)),
                               trace=False)
    total = np.float64(0.0)
    for c in range(N_CORES):
        total += np.float64(res.results[c]["osum"][0, 0])
    mean = np.float32(total / (B * M_TOTAL))
    return np.asarray(mean, dtype=np.float32)


if __name__ == "__main__":
    rng = np.random.default_rng(0)
    pred = rng.random((B, M_TOTAL, 3), dtype=np.float32)
    gt = rng.random((B, N_GT, 3), dtype=np.float32)
    out = kernel(pred, gt)
    print("kernel out:", out)
